# revision 24
# baseline (speedup 1.0000x reference)
"""Bahdanau-attention kernel for Trainium2 (8 NeuronCores, data-parallel over batch).

Computes, for each batch b:
    q[b]    = v * (W_w @ prev[b] + W_b + U_b)            (host, tiny)
    U'      = v[:, None] * U_w                            (host, tiny)
    e[b,t]  = sum_h relu(q[b,h] + (U' @ enc[b,t])_h)      (device)
    alpha   = softmax(e[b, :])                            (device)
    out[b]  = sum_t alpha[t] * enc[b,t,:]                 (device)

The v>0 fold is exact: v_h * relu(x_h) == relu(v_h * x_h) for v_h >= 0.

Device strategy (per core: 4 batches; the host casts enc to fp16 before
upload, so each core streams its [4, 4096, 1024] slice from HBM exactly
once as 32 MB of fp16 — fp16's 10-bit mantissa matches the tf32-grade
rounding f32r gives on HW, at half the byte width, 1 cyc/row PE transposes,
and FWL fast weight loads; fp16 in HBM also halves the read traffic and
keeps the loads on plain HWDGE instead of the SWDGE cast path):
  - One 2 MB dma_start per 8-tile group (4 per batch): amortizes the DMA
    fixed costs that killed the per-tile-DMA version (755 us -> 394 us).
  - enc group tiles [128, 8, 1024] fp16 stay SBUF-resident for the batch;
    the 8-buffer pool double-buffers a full batch ahead.
  - PE transposes each tile chunk-wise to [c, t] (fp16, PSUM), DVE copies
    the result to SBUF.
  - U-matmul in fp16 accumulates [t=128, h=256] in fp32 PSUM; the q bias is
    broadcast once per batch (ones-row matmul) and added per tile on DVE,
    keeping 32 LDW+MM pairs per batch off the bottleneck PE pipeline.
  - ACT fused relu+row-reduce produces the energy column per tile.
  - Exact fp32 two-level softmax: per-partition max shift via the ACT bias,
    then a one-partition fixup; cross-partition gather/scatter rides PE
    transposes / a K=1 matmul.
  - Pass-2 weighted sum: M=1 matmuls use 1 of 128 PE columns, so 4 tiles
    run concurrently in distinct column groups (tile_position), partials
    land at partitions 0/32/64/96 and a one-hot select-sum matmul combines
    them (394 us -> ~310 us with the bias change).
Rejected alternatives (measured): HWDGE fp32 no-cast DMA is slower (more
SBUF-side bytes; combined in+out budget ~300 GB/s here); an xbar
dma_start_transpose architecture (transposed loads + DVE pass-2) has a
slower DMA floor (~142 GB/s/core for transposed reads) and wedged the
device when combined with DRAM-pool bounces in this runtime.

Toolchain notes: the module is built as a Bacc (not raw Bass) so multi-wait
instructions get legalized into event semaphores and the walrus single-wait
LDWEIGHTS limit is respected. Matmul inputs must not mix 16/32-bit dtypes;
the softmax's per-partition max is rounded to fp16 FIRST and the rounded
value used in both exponents so z'*g composes exactly.
"""

import sys

import numpy as np

sys.path.insert(0, "/opt/trn_rl_repo")

import concourse.bacc as bacc
import concourse.mybir as mybir
import concourse.tile as tile
from concourse.bass import ts
from concourse.bass_utils import run_bass_kernel_spmd
from concourse.masks import make_identity

B, T, C, H, D = 32, 4096, 1024, 256, 512
NCORES = 8
BPC = B // NCORES  # batches per core

F32 = mybir.dt.float32
F32R = mybir.dt.float32r
F16 = mybir.dt.float16
BF16 = mybir.dt.bfloat16

P = 128            # partitions / t-tile size
CK = C // P        # 8 c-chunks per tile
NT = T // P        # 32 t-tiles per batch
GT = 8             # t-tiles per DMA group (one 2 MB fp16 dma_start each)


def build_bass(bpc: int = BPC, n_tiles: int = NT, repeat: int = 1):
    nc = bacc.Bacc(target_bir_lowering=False, trn_type="TRN2")

    # enc arrives in HBM already cast to fp16 by the host: halves the HBM
    # read traffic vs fp32 and lets the loads ride plain HWDGE (no SWDGE
    # cast path, no serialized Q7 descriptor generation)
    enc = nc.dram_tensor("enc", [bpc, n_tiles * P, C], F16, kind="ExternalInput")
    # q as columns [p, 2*b + hh] = q[b, hh*128 + p]: with the U-matmul in
    # [h, t] layout the q-bias is per-PARTITION, so ACT's bias port adds it
    qcol = nc.dram_tensor("qcol", [P, 2 * bpc], F32, kind="ExternalInput")
    # U' transposed, pre-arranged host-side as [p, chunk, h] with c = chunk*128 + p
    ut = nc.dram_tensor("ut", [P, CK, H], F32, kind="ExternalInput")
    out = nc.dram_tensor("out", [bpc, C], F32, kind="ExternalOutput")

    ngrp = n_tiles // GT  # DMA groups per batch

    enc_ap = enc.ap()
    out_ap = out.ap()

    with tile.TileContext(nc) as tc:
        with (
            tc.tile_pool(name="singles", bufs=1) as singles,
            tc.tile_pool(name="enc_pool", bufs=2 * ngrp) as enc_pool,
            tc.tile_pool(name="encT_pool", bufs=3) as encT_pool,
            tc.tile_pool(name="relu_pool", bufs=3) as relu_pool,
            tc.tile_pool(name="batch_pool", bufs=2) as batch_pool,
            tc.tile_pool(name="small_pool", bufs=2) as small_pool,
            tc.tile_pool(name="outst_pool", bufs=2) as outst_pool,
            tc.tile_pool(name="ps_tp", bufs=2, space="PSUM") as ps_tp,
            tc.tile_pool(name="ps_um", bufs=2, space="PSUM") as ps_um,
            tc.tile_pool(name="ps_c", bufs=1, space="PSUM") as ps_c,
            tc.tile_pool(name="ps_s", bufs=1, space="PSUM") as ps_s,
        ):
            # --- constants, all funneled through DVE so PE sees one clock ---
            ident_stage = singles.tile([P, P], F32)
            make_identity(nc, ident_stage)
            ut_stage = singles.tile([P, CK, H], F32)
            nc.gpsimd.dma_start(out=ut_stage, in_=ut.ap())
            qcol_s = singles.tile([P, 2 * bpc], F32)
            nc.gpsimd.dma_start(out=qcol_s, in_=qcol.ap())

            ones_row_f = singles.tile([1, P], F32)
            nc.vector.memset(ones_row_f, 1.0)
            ones_row = singles.tile([1, P], F16)
            nc.vector.tensor_copy(ones_row, ones_row_f)
            ones_col_f = singles.tile([P, 1], F32)
            nc.vector.memset(ones_col_f, 1.0)
            ones_col = singles.tile([P, 1], F16)
            nc.vector.tensor_copy(ones_col, ones_col_f)
            ut_s = singles.tile([P, CK, H], F16)
            nc.vector.tensor_copy(ut_s, ut_stage)
            ident_h = singles.tile([P, P], F16)
            nc.vector.tensor_copy(ident_h, ident_stage)
            # one-hot column selecting partitions 0/32/64/96 (pass-2 combine)
            sel4 = singles.tile([P, 1], F32)
            nc.vector.memset(sel4, 0.0)
            for cg in range(0, P, 32):
                nc.vector.memset(sel4[cg : cg + 1, :], 1.0)

            def batches():
              for b in range(bpc):
                # ---------------- pass 1: energies ----------------
                # One 2 MB dma_start per 8-tile group: amortizes the DMA
                # fixed costs over 8 tiles instead of paying them per tile.
                # The U-matmul runs in [h, t] layout over 4-tile groups:
                # stationary = ut chunk [c, h-half], moving = encT [c, t=512]
                # — half the U-MM instruction count of the per-tile [t, h]
                # form, and the q bias rides ACT's per-partition bias port.
                grp_tiles = []
                e_buf = batch_pool.tile([P, n_tiles], F32, tag="ebuf")
                enc_b = enc_ap[b].rearrange("(g j p) c -> g p j c", g=ngrp, j=GT, p=P)
                for g in range(ngrp):
                    enc_g = enc_pool.tile([P, GT, C], F16, tag="enc")
                    nc.sync.dma_start(out=enc_g, in_=enc_b[g])
                    grp_tiles.append(enc_g)
                    for u in range(GT // 4):
                        # transpose the group's 4 tiles chunk-wise into one
                        # [c, k, t=512] buffer (one DVE copy per tile)
                        encTg = encT_pool.tile([P, CK, 4 * P], F16, tag="encT")
                        for jj in range(4):
                            enc_t = enc_g[:, u * 4 + jj, :]
                            tp = ps_tp.tile([P, C], F16, tag="tp")
                            for k in range(CK):
                                nc.tensor.transpose(
                                    tp[:, ts(k, P)], enc_t[:, ts(k, P)], ident_h
                                )
                            nc.vector.tensor_copy(
                                encTg[:, :, ts(jj, P)],
                                tp.rearrange("p (k t) -> p k t", k=CK),
                            )
                        # U-matmul per h-half: psum[h, t] = sum_c ut^T encT,
                        # then ACT relu with the per-partition q bias
                        relu_halves = []
                        for hh in range(2):
                            um = ps_um.tile([P, 4 * P], F32, tag="um")
                            for k in range(CK):
                                nc.tensor.matmul(
                                    um,
                                    ut_s[:, k, ts(hh, P)],
                                    encTg[:, k, :],
                                    start=(k == 0),
                                    stop=(k == CK - 1),
                                )
                            relu_h = relu_pool.tile([P, 4 * P], F16, tag="relu")
                            nc.scalar.activation(
                                out=relu_h,
                                in_=um,
                                func=mybir.ActivationFunctionType.Relu,
                                bias=qcol_s[:, 2 * b + hh : 2 * b + hh + 1],
                            )
                            relu_halves.append(relu_h)
                        # e[t] = sum_h relu[h, t]: ones-column matmuls per
                        # tile, both halves accumulating into one psum column
                        e_ps4 = ps_tp.tile([P, 4], F32, tag="tp")
                        for jj in range(4):
                            for hh in range(2):
                                nc.tensor.matmul(
                                    e_ps4[:, jj : jj + 1],
                                    relu_halves[hh][:, ts(jj, P)],
                                    ones_col,
                                    start=(hh == 0),
                                    stop=(hh == 1),
                                )
                        j0 = g * GT + u * 4
                        nc.vector.tensor_copy(e_buf[:, j0 : j0 + 4], e_ps4)

                # ------- softmax (exact fp32, two-level, PE transposes) -------
                # z'[p,j] = exp(e[p,j] - mp[p]) with the per-partition max mp
                # (ACT bias is per-partition, so no broadcast needed), then a
                # one-partition fixup computes g[p] = exp(mp[p]-M)/S and
                # alpha = z' * g  ==  exp(e-M)/S exactly. Cross-partition
                # gather/scatter rides the PE transpose (sub-us) instead of
                # SBUF->SBUF DMA (~1.5us fixed each).
                ms = small_pool.tile([P, 2], F32, tag="ms")
                nc.vector.tensor_reduce(
                    ms[:, 0:1], e_buf, axis=mybir.AxisListType.X,
                    op=mybir.AluOpType.max,
                )
                ms_r = small_pool.tile([P, 2], F16, tag="ms_r")
                nc.vector.tensor_copy(ms_r[:, 0:1], ms[:, 0:1])
                mpneg = small_pool.tile([P, 1], F32, tag="mpneg")
                nc.vector.tensor_scalar_mul(mpneg, ms_r[:, 0:1], -1.0)
                z = batch_pool.tile([P, n_tiles], F32, tag="z")
                nc.scalar.activation(
                    out=z,
                    in_=e_buf,
                    func=mybir.ActivationFunctionType.Exp,
                    bias=mpneg,
                    accum_out=ms[:, 1:2],
                )
                # gather each column onto partition 0 via PE transposes
                # (SBUF partition offsets must be 32-aligned, so the two
                # columns can't share one [2, P] tile)
                nc.vector.tensor_copy(ms_r[:, 1:2], ms[:, 1:2])
                mrow_ps = ps_tp.tile([1, P], F16, tag="tp")
                nc.tensor.transpose(mrow_ps, ms_r[:, 0:1], ident_h)
                srow_ps = ps_tp.tile([1, P], F16, tag="tp")
                nc.tensor.transpose(srow_ps, ms_r[:, 1:2], ident_h)
                mrow = small_pool.tile([1, P], F32, tag="mrow")
                nc.vector.tensor_copy(mrow, mrow_ps)
                srow = small_pool.tile([1, P], F32, tag="srow")
                nc.vector.tensor_copy(srow, srow_ps)
                mtot = small_pool.tile([1, 1], F32, tag="mtot")
                nc.vector.tensor_reduce(
                    mtot, mrow, axis=mybir.AxisListType.X, op=mybir.AluOpType.max
                )
                mtneg = small_pool.tile([1, 1], F32, tag="mtneg")
                nc.vector.tensor_scalar_mul(mtneg, mtot, -1.0)
                grow = small_pool.tile([1, P], F32, tag="grow")
                nc.scalar.activation(
                    out=grow,
                    in_=mrow,
                    func=mybir.ActivationFunctionType.Exp,
                    bias=mtneg,
                )
                wrow = small_pool.tile([1, P], F32, tag="wrow")
                nc.vector.tensor_mul(wrow, grow, srow)
                stot = small_pool.tile([1, 1], F32, tag="stot")
                nc.vector.tensor_reduce(
                    stot, wrow, axis=mybir.AxisListType.X, op=mybir.AluOpType.add
                )
                rec = small_pool.tile([1, 1], F32, tag="rec")
                nc.vector.reciprocal(rec, stot)
                gsrow = small_pool.tile([1, P], F32, tag="gsrow")
                nc.vector.tensor_scalar_mul(gsrow, grow, rec)
                gsrow_r = small_pool.tile([1, P], F16, tag="gsrow_r")
                nc.vector.tensor_copy(gsrow_r, gsrow)
                # scatter g[p]/S back to one element per partition via a
                # K=1 matmul: out[p, 0] = gsrow[p] * 1
                gscol_ps = ps_tp.tile([P, 32], F32, tag="tp")
                nc.tensor.matmul(
                    gscol_ps, gsrow_r, ones_row[:, 0:32], start=True, stop=True
                )
                gscol = small_pool.tile([P, 1], F32, tag="gscol")
                nc.vector.tensor_copy(gscol, gscol_ps[:, 0:1])
                alpha = batch_pool.tile([P, n_tiles], F16, tag="alpha")
                nc.vector.tensor_scalar_mul(alpha, z, gscol)

                # ---------------- pass 2: weighted sum ----------------
                # 4-way column-group packing: M=1 matmuls use 1 of 128 PE
                # columns, so run 4 tiles concurrently in distinct col groups
                # (tile_position), partials landing at partitions 0/32/64/96.
                cps = ps_c.tile([P, 2, D], F32, tag="cps")
                for j in range(n_tiles):
                    enc_t = grp_tiles[j // GT][:, j % GT, :]
                    cg = 32 * (j % 4)
                    for h in range(2):
                        nc.tensor.matmul(
                            cps[cg : cg + 1, h, :],
                            alpha[:, j : j + 1],
                            enc_t[:, ts(h, D)],
                            start=(j < 4),
                            stop=(j >= n_tiles - 4),
                            tile_position=(0, cg),
                        )
                # combine the 4 partials: full-lane PSUM->SBUF copy, then a
                # one-hot select-sum matmul collapses partitions 0/32/64/96
                # (DVE cannot combine across partitions)
                c_all = outst_pool.tile([P, 2, D], F32, tag="call")
                nc.vector.tensor_copy(c_all, cps)
                csum = ps_s.tile([1, 2, D], F32, tag="csum")
                for h in range(2):
                    nc.tensor.matmul(
                        csum[:, h, :], sel4, c_all[:, h, :], start=True, stop=True
                    )
                c_st = outst_pool.tile([1, C], F32, tag="cst")
                nc.vector.tensor_copy(c_st, csum.rearrange("p a b -> p (a b)"))
                nc.sync.dma_start(out=out_ap[b : b + 1, :], in_=c_st)

            if repeat == 1:
                batches()
            else:
                with tc.For_i(0, repeat, 1):
                    batches()

    return nc


_NC_CACHE: dict = {}


def _get_nc(bpc=BPC, n_tiles=NT):
    key = (bpc, n_tiles)
    if key not in _NC_CACHE:
        nc = build_bass(bpc, n_tiles)
        if not nc.is_finalized():
            nc.finalize()
        _NC_CACHE[key] = nc
    return _NC_CACHE[key]


def _host_prep(previous_decoder_hidden_state, W_w, W_b, U_w, U_b, v):
    prev = np.asarray(previous_decoder_hidden_state, dtype=np.float32)[:, 0, :]
    W_w = np.asarray(W_w, dtype=np.float32)
    U_w = np.asarray(U_w, dtype=np.float32)
    v = np.asarray(v, dtype=np.float32)
    bias = np.asarray(W_b, dtype=np.float32) + np.asarray(U_b, dtype=np.float32)
    q_all = (v[None, :] * (prev @ W_w.T + bias)).astype(np.float32)  # [B, H]
    up = (v[:, None] * U_w).astype(np.float32)  # [H, C]
    # ut_host[p, k, h] = up.T[k*128 + p, h]
    ut_host = np.ascontiguousarray(up.T.reshape(CK, P, H).transpose(1, 0, 2))
    return q_all, ut_host


def make_in_maps(inputs):
    enc16 = np.asarray(
        inputs["encoder_final_hidden_layers"], dtype=np.float32
    ).astype(np.float16)
    q_all, ut_host = _host_prep(
        inputs["previous_decoder_hidden_state"],
        inputs["W_w"],
        inputs["W_b"],
        inputs["U_w"],
        inputs["U_b"],
        inputs["v"],
    )
    in_maps = []
    for i in range(NCORES):
        sl = slice(i * BPC, (i + 1) * BPC)
        # qcol[p, 2*b + hh] = q_all[b, hh*128 + p]
        qc = np.ascontiguousarray(
            q_all[sl].reshape(BPC, 2, P).transpose(2, 0, 1).reshape(P, 2 * BPC)
        )
        in_maps.append(
            {
                "enc": np.ascontiguousarray(enc16[sl]),
                "qcol": qc,
                "ut": ut_host,
            }
        )
    return in_maps


def kernel(**inputs) -> np.ndarray:
    in_maps = make_in_maps(inputs)
    nc = _get_nc()
    try:
        res = run_bass_kernel_spmd(nc, in_maps, core_ids=list(range(NCORES)))
    except Exception:
        # a previously crashed run can leave a core wedged
        # (NRT_EXEC_UNIT_UNRECOVERABLE); one retry recovers
        res = run_bass_kernel_spmd(nc, in_maps, core_ids=list(range(NCORES)))
    return np.concatenate([r["out"] for r in res.results], axis=0)


if __name__ == "__main__":
    nc = build_bass()
    print("built ok")



# revision 25
# speedup vs baseline: 1.5020x; 1.5020x over previous
"""Bahdanau-attention kernel for Trainium2 (8 NeuronCores, data-parallel over batch).

Computes, for each batch b:
    q[b]    = v * (W_w @ prev[b] + W_b + U_b)            (host, tiny)
    U'      = v[:, None] * U_w                            (host, tiny)
    e[b,t]  = sum_h relu(q[b,h] + (U' @ enc[b,t])_h)      (device)
    alpha   = softmax(e[b, :])                            (device)
    out[b]  = sum_t alpha[t] * enc[b,t,:]                 (device)

The v>0 fold is exact: v_h * relu(x_h) == relu(v_h * x_h) for v_h >= 0.

Device strategy (per core: 4 batches; the host casts enc to fp16 before
upload, so each core streams its [4, 4096, 1024] slice from HBM exactly
once as 32 MB of fp16 — fp16's 10-bit mantissa matches the tf32-grade
rounding f32r gives on HW, at half the byte width, 1 cyc/row PE transposes,
and FWL fast weight loads; fp16 in HBM also halves the read traffic and
keeps the loads on plain HWDGE instead of the SWDGE cast path):
  - One 2 MB dma_start per 8-tile group (4 per batch): amortizes the DMA
    fixed costs that killed the per-tile-DMA version (755 us -> 394 us).
  - enc group tiles [128, 8, 1024] fp16 stay SBUF-resident for the batch;
    the 8-buffer pool double-buffers a full batch ahead.
  - PE transposes each tile chunk-wise to [c, t] (fp16, PSUM), DVE copies
    the result to SBUF.
  - U-matmul in fp16 accumulates [t=128, h=256] in fp32 PSUM; the q bias is
    broadcast once per batch (ones-row matmul) and added per tile on DVE,
    keeping 32 LDW+MM pairs per batch off the bottleneck PE pipeline.
  - ACT fused relu+row-reduce produces the energy column per tile.
  - Exact fp32 two-level softmax: per-partition max shift via the ACT bias,
    then a one-partition fixup; cross-partition gather/scatter rides PE
    transposes / a K=1 matmul.
  - Pass-2 weighted sum: M=1 matmuls use 1 of 128 PE columns, so 4 tiles
    run concurrently in distinct column groups (tile_position), partials
    land at partitions 0/32/64/96 and a one-hot select-sum matmul combines
    them (394 us -> ~310 us with the bias change).
Rejected alternatives (measured): HWDGE fp32 no-cast DMA is slower (more
SBUF-side bytes; combined in+out budget ~300 GB/s here); an xbar
dma_start_transpose architecture (transposed loads + DVE pass-2) has a
slower DMA floor (~142 GB/s/core for transposed reads) and wedged the
device when combined with DRAM-pool bounces in this runtime.

Toolchain notes: the module is built as a Bacc (not raw Bass) so multi-wait
instructions get legalized into event semaphores and the walrus single-wait
LDWEIGHTS limit is respected. Matmul inputs must not mix 16/32-bit dtypes;
the softmax's per-partition max is rounded to fp16 FIRST and the rounded
value used in both exponents so z'*g composes exactly.
"""

import sys

import numpy as np

sys.path.insert(0, "/opt/trn_rl_repo")

import concourse.bacc as bacc
import concourse.mybir as mybir
import concourse.tile as tile
from concourse.bass import ts
from concourse.bass_utils import run_bass_kernel_spmd
from concourse.masks import make_identity

B, T, C, H, D = 32, 4096, 1024, 256, 512
NCORES = 8
BPC = B // NCORES  # batches per core

F32 = mybir.dt.float32
F32R = mybir.dt.float32r
F16 = mybir.dt.float16
BF16 = mybir.dt.bfloat16

P = 128            # partitions / t-tile size
CK = C // P        # 8 c-chunks per tile
NT = T // P        # 32 t-tiles per batch
GT = 8             # t-tiles per DMA group (one 2 MB fp16 dma_start each)


def build_bass(bpc: int = BPC, n_tiles: int = NT, repeat: int = 1):
    nc = bacc.Bacc(target_bir_lowering=False, trn_type="TRN2")

    # enc arrives in HBM already cast to fp16 by the host: halves the HBM
    # read traffic vs fp32 and lets the loads ride plain HWDGE (no SWDGE
    # cast path, no serialized Q7 descriptor generation)
    enc = nc.dram_tensor("enc", [bpc, n_tiles * P, C], F16, kind="ExternalInput")
    # q rows packed on one partition: [1, bpc*H]
    qrow = nc.dram_tensor("qrow", [1, bpc * H], F32, kind="ExternalInput")
    # U' transposed, pre-arranged host-side as [p, chunk, h] with c = chunk*128 + p
    ut = nc.dram_tensor("ut", [P, CK, H], F32, kind="ExternalInput")
    out = nc.dram_tensor("out", [bpc, C], F32, kind="ExternalOutput")

    ngrp = n_tiles // GT  # DMA groups per batch

    enc_ap = enc.ap()
    out_ap = out.ap()

    with tile.TileContext(nc) as tc:
        with (
            tc.tile_pool(name="singles", bufs=1) as singles,
            tc.tile_pool(name="enc_pool", bufs=2 * ngrp) as enc_pool,
            tc.tile_pool(name="encT_pool", bufs=3) as encT_pool,
            tc.tile_pool(name="relu_pool", bufs=3) as relu_pool,
            tc.tile_pool(name="batch_pool", bufs=2) as batch_pool,
            tc.tile_pool(name="small_pool", bufs=2) as small_pool,
            tc.tile_pool(name="outst_pool", bufs=2) as outst_pool,
            tc.tile_pool(name="ps_tp", bufs=2, space="PSUM") as ps_tp,
            tc.tile_pool(name="ps_um", bufs=2, space="PSUM") as ps_um,
            tc.tile_pool(name="ps_c", bufs=1, space="PSUM") as ps_c,
            tc.tile_pool(name="ps_s", bufs=1, space="PSUM") as ps_s,
        ):
            # --- constants, all funneled through DVE so PE sees one clock ---
            ident_stage = singles.tile([P, P], F32)
            make_identity(nc, ident_stage)
            ut_stage = singles.tile([P, CK, H], F32)
            nc.gpsimd.dma_start(out=ut_stage, in_=ut.ap())
            q_stage = singles.tile([1, bpc * H], F32)
            nc.gpsimd.dma_start(out=q_stage, in_=qrow.ap())

            ones_row_f = singles.tile([1, P], F32)
            nc.vector.memset(ones_row_f, 1.0)
            ones_row = singles.tile([1, P], F16)
            nc.vector.tensor_copy(ones_row, ones_row_f)
            q_s = singles.tile([1, bpc * H], F16)
            nc.vector.tensor_copy(q_s, q_stage)
            ut_s = singles.tile([P, CK, H], F16)
            nc.vector.tensor_copy(ut_s, ut_stage)
            ident_h = singles.tile([P, P], F16)
            nc.vector.tensor_copy(ident_h, ident_stage)
            # one-hot column selecting partitions 0/32/64/96 (pass-2 combine)
            sel4 = singles.tile([P, 1], F32)
            nc.vector.memset(sel4, 0.0)
            for cg in range(0, P, 32):
                nc.vector.memset(sel4[cg : cg + 1, :], 1.0)

            def batches():
              for b in range(bpc):
                # ---------------- pass 1: energies ----------------
                # One 2 MB dma_start per 8-tile group: amortizes the DMA
                # fixed costs over 8 tiles instead of paying them per tile.
                grp_tiles = []
                e_buf = batch_pool.tile([P, n_tiles], F32, tag="ebuf")
                # q broadcast to all partitions once per batch; the per-tile
                # q-bias then rides a cheap DVE add instead of a PE matmul
                qb_ps = ps_um.tile([P, H], F32, tag="um")
                nc.tensor.matmul(
                    qb_ps,
                    ones_row,
                    q_s[:, b * H : (b + 1) * H],
                    start=True,
                    stop=True,
                )
                q_bc = batch_pool.tile([P, H], F32, tag="qbc")
                nc.vector.tensor_copy(q_bc, qb_ps)
                enc_b = enc_ap[b].rearrange("(g j p) c -> g p j c", g=ngrp, j=GT, p=P)
                for g in range(ngrp):
                    enc_g = enc_pool.tile([P, GT, C], F16, tag="enc")
                    nc.sync.dma_start(out=enc_g, in_=enc_b[g])
                    grp_tiles.append(enc_g)
                    for jj in range(GT):
                        j = g * GT + jj
                        enc_t = enc_g[:, jj, :]

                        # transpose 8 chunks [t,c]->[c,t] into one PSUM bank,
                        # then one DVE copy [128, 1024] to SBUF
                        encT = encT_pool.tile([P, C], F16, tag="encT")
                        tp = ps_tp.tile([P, C], F16, tag="tp")
                        for k in range(CK):
                            nc.tensor.transpose(
                                tp[:, ts(k, P)], enc_t[:, ts(k, P)], ident_h
                            )
                        nc.vector.tensor_copy(encT, tp)

                        # U-matmul: psum[t, h] = sum_c encT[c,t]^T ut[c,h]
                        um = ps_um.tile([P, H], F32, tag="um")
                        for k in range(CK):
                            nc.tensor.matmul(
                                um,
                                encT[:, ts(k, P)],
                                ut_s[:, k, :],
                                start=(k == 0),
                                stop=(k == CK - 1),
                            )

                        # add q bias (DVE), then e[t] = sum_h relu(.) on ACT
                        relu_in = relu_pool.tile([P, H], F32, tag="reluin")
                        nc.vector.tensor_add(relu_in, um, q_bc)
                        relu_sc = relu_pool.tile([P, H], BF16, tag="relu")
                        nc.scalar.activation(
                            out=relu_sc,
                            in_=relu_in,
                            func=mybir.ActivationFunctionType.Relu,
                            accum_out=e_buf[:, j : j + 1],
                        )

                # ------- softmax (exact fp32, two-level, PE transposes) -------
                # z'[p,j] = exp(e[p,j] - mp[p]) with the per-partition max mp
                # (ACT bias is per-partition, so no broadcast needed), then a
                # one-partition fixup computes g[p] = exp(mp[p]-M)/S and
                # alpha = z' * g  ==  exp(e-M)/S exactly. Cross-partition
                # gather/scatter rides the PE transpose (sub-us) instead of
                # SBUF->SBUF DMA (~1.5us fixed each).
                ms = small_pool.tile([P, 2], F32, tag="ms")
                nc.vector.tensor_reduce(
                    ms[:, 0:1], e_buf, axis=mybir.AxisListType.X,
                    op=mybir.AluOpType.max,
                )
                ms_r = small_pool.tile([P, 2], F16, tag="ms_r")
                nc.vector.tensor_copy(ms_r[:, 0:1], ms[:, 0:1])
                mpneg = small_pool.tile([P, 1], F32, tag="mpneg")
                nc.vector.tensor_scalar_mul(mpneg, ms_r[:, 0:1], -1.0)
                z = batch_pool.tile([P, n_tiles], F32, tag="z")
                nc.scalar.activation(
                    out=z,
                    in_=e_buf,
                    func=mybir.ActivationFunctionType.Exp,
                    bias=mpneg,
                    accum_out=ms[:, 1:2],
                )
                # gather each column onto partition 0 via PE transposes
                # (SBUF partition offsets must be 32-aligned, so the two
                # columns can't share one [2, P] tile)
                nc.vector.tensor_copy(ms_r[:, 1:2], ms[:, 1:2])
                mrow_ps = ps_tp.tile([1, P], F16, tag="tp")
                nc.tensor.transpose(mrow_ps, ms_r[:, 0:1], ident_h)
                srow_ps = ps_tp.tile([1, P], F16, tag="tp")
                nc.tensor.transpose(srow_ps, ms_r[:, 1:2], ident_h)
                mrow = small_pool.tile([1, P], F32, tag="mrow")
                nc.vector.tensor_copy(mrow, mrow_ps)
                srow = small_pool.tile([1, P], F32, tag="srow")
                nc.vector.tensor_copy(srow, srow_ps)
                mtot = small_pool.tile([1, 1], F32, tag="mtot")
                nc.vector.tensor_reduce(
                    mtot, mrow, axis=mybir.AxisListType.X, op=mybir.AluOpType.max
                )
                mtneg = small_pool.tile([1, 1], F32, tag="mtneg")
                nc.vector.tensor_scalar_mul(mtneg, mtot, -1.0)
                grow = small_pool.tile([1, P], F32, tag="grow")
                nc.scalar.activation(
                    out=grow,
                    in_=mrow,
                    func=mybir.ActivationFunctionType.Exp,
                    bias=mtneg,
                )
                wrow = small_pool.tile([1, P], F32, tag="wrow")
                nc.vector.tensor_mul(wrow, grow, srow)
                stot = small_pool.tile([1, 1], F32, tag="stot")
                nc.vector.tensor_reduce(
                    stot, wrow, axis=mybir.AxisListType.X, op=mybir.AluOpType.add
                )
                rec = small_pool.tile([1, 1], F32, tag="rec")
                nc.vector.reciprocal(rec, stot)
                gsrow = small_pool.tile([1, P], F32, tag="gsrow")
                nc.vector.tensor_scalar_mul(gsrow, grow, rec)
                gsrow_r = small_pool.tile([1, P], F16, tag="gsrow_r")
                nc.vector.tensor_copy(gsrow_r, gsrow)
                # scatter g[p]/S back to one element per partition via a
                # K=1 matmul: out[p, 0] = gsrow[p] * 1
                gscol_ps = ps_tp.tile([P, 32], F32, tag="tp")
                nc.tensor.matmul(
                    gscol_ps, gsrow_r, ones_row[:, 0:32], start=True, stop=True
                )
                gscol = small_pool.tile([P, 1], F32, tag="gscol")
                nc.vector.tensor_copy(gscol, gscol_ps[:, 0:1])
                alpha = batch_pool.tile([P, n_tiles], F16, tag="alpha")
                nc.vector.tensor_scalar_mul(alpha, z, gscol)

                # ---------------- pass 2: weighted sum ----------------
                # 4-way column-group packing: M=1 matmuls use 1 of 128 PE
                # columns, so run 4 tiles concurrently in distinct col groups
                # (tile_position), partials landing at partitions 0/32/64/96.
                cps = ps_c.tile([P, 2, D], F32, tag="cps")
                for j in range(n_tiles):
                    enc_t = grp_tiles[j // GT][:, j % GT, :]
                    cg = 32 * (j % 4)
                    for h in range(2):
                        nc.tensor.matmul(
                            cps[cg : cg + 1, h, :],
                            alpha[:, j : j + 1],
                            enc_t[:, ts(h, D)],
                            start=(j < 4),
                            stop=(j >= n_tiles - 4),
                            tile_position=(0, cg),
                        )
                # combine the 4 partials: full-lane PSUM->SBUF copy, then a
                # one-hot select-sum matmul collapses partitions 0/32/64/96
                # (DVE cannot combine across partitions)
                c_all = outst_pool.tile([P, 2, D], F32, tag="call")
                nc.vector.tensor_copy(c_all, cps)
                csum = ps_s.tile([1, 2, D], F32, tag="csum")
                for h in range(2):
                    nc.tensor.matmul(
                        csum[:, h, :], sel4, c_all[:, h, :], start=True, stop=True
                    )
                c_st = outst_pool.tile([1, C], F32, tag="cst")
                nc.vector.tensor_copy(c_st, csum.rearrange("p a b -> p (a b)"))
                nc.sync.dma_start(out=out_ap[b : b + 1, :], in_=c_st)

            if repeat == 1:
                batches()
            else:
                with tc.For_i(0, repeat, 1):
                    batches()

    return nc


_NC_CACHE: dict = {}


def _get_nc(bpc=BPC, n_tiles=NT):
    key = (bpc, n_tiles)
    if key not in _NC_CACHE:
        nc = build_bass(bpc, n_tiles)
        if not nc.is_finalized():
            nc.finalize()
        _NC_CACHE[key] = nc
    return _NC_CACHE[key]


def _host_prep(previous_decoder_hidden_state, W_w, W_b, U_w, U_b, v):
    prev = np.asarray(previous_decoder_hidden_state, dtype=np.float32)[:, 0, :]
    W_w = np.asarray(W_w, dtype=np.float32)
    U_w = np.asarray(U_w, dtype=np.float32)
    v = np.asarray(v, dtype=np.float32)
    bias = np.asarray(W_b, dtype=np.float32) + np.asarray(U_b, dtype=np.float32)
    q_all = (v[None, :] * (prev @ W_w.T + bias)).astype(np.float32)  # [B, H]
    up = (v[:, None] * U_w).astype(np.float32)  # [H, C]
    # ut_host[p, k, h] = up.T[k*128 + p, h]
    ut_host = np.ascontiguousarray(up.T.reshape(CK, P, H).transpose(1, 0, 2))
    return q_all, ut_host


def make_in_maps(inputs):
    enc16 = np.asarray(
        inputs["encoder_final_hidden_layers"], dtype=np.float32
    ).astype(np.float16)
    q_all, ut_host = _host_prep(
        inputs["previous_decoder_hidden_state"],
        inputs["W_w"],
        inputs["W_b"],
        inputs["U_w"],
        inputs["U_b"],
        inputs["v"],
    )
    in_maps = []
    for i in range(NCORES):
        sl = slice(i * BPC, (i + 1) * BPC)
        in_maps.append(
            {
                "enc": np.ascontiguousarray(enc16[sl]),
                "qrow": np.ascontiguousarray(q_all[sl].reshape(1, BPC * H)),
                "ut": ut_host,
            }
        )
    return in_maps


def kernel(**inputs) -> np.ndarray:
    in_maps = make_in_maps(inputs)
    nc = _get_nc()
    try:
        res = run_bass_kernel_spmd(nc, in_maps, core_ids=list(range(NCORES)))
    except Exception:
        # a previously crashed run can leave a core wedged
        # (NRT_EXEC_UNIT_UNRECOVERABLE); one retry recovers
        res = run_bass_kernel_spmd(nc, in_maps, core_ids=list(range(NCORES)))
    return np.concatenate([r["out"] for r in res.results], axis=0)


if __name__ == "__main__":
    nc = build_bass()
    print("built ok")

